# revision 3
# baseline (speedup 1.0000x reference)
"""Leaky-integrator scan out[:,t] = out[:,t-1]*sigmoid(w) + X[:,t] on 8 trn2 cores.

Reformulated as a lower-triangular Toeplitz matmul over the time dim:
    out[b] = L @ X[b],  L[t, s] = a^(t-s) (t >= s),  a = sigmoid(w)
with T=256 split into two 128-row blocks. By Toeplitz structure L00 == L11
(lower-tri powers) and L10[i, j] = a^(128+i-j), so only two stationary
128x128 weight matrices are needed on the TensorEngine.

Numerics: X is split on the host into X_hi + X_lo (bf16 pair, same total
HBM bytes as the original f32) and both halves are accumulated into the
same fp32 PSUM group, giving near-f32 accuracy at bf16 matmul throughput.

Sharding: data-parallel over batch B (16 / 8 cores = 2 per core), which
keeps host-side shard slices contiguous and needs no cross-core traffic.
"""

import math
import os
import sys

import numpy as np

for _p in ("/opt/trn_rl_repo", "/root/.axon_site/_ro/trn_rl_repo"):
    if os.path.isdir(_p) and _p not in sys.path:
        sys.path.insert(0, _p)

import ml_dtypes

import concourse.bass as bass
import concourse.mybir as mybir
from concourse import bacc
from concourse.tile import TileContext
from concourse.bass_utils import run_bass_kernel_spmd

B, T, N = 16, 256, 32768
N_CORES = 8
B_PER = B // N_CORES  # 2
P = 128               # partitions / time-block size
TK = 2048             # free-dim (feature) tile width
MM = 512              # matmul moving free dim (one PSUM bank of fp32)
NJ = N // TK          # 16 feature tiles per batch
SL = TK // MM         # 4 matmul slices per feature tile

BF16 = mybir.dt.bfloat16
F32 = mybir.dt.float32

_compiled_nc = None


def _build_nc():
    """Build + compile the SPMD Bass graph (identical on all 8 cores)."""
    nc = bacc.Bacc("TRN2", target_bir_lowering=False, debug=False,
                   num_devices=N_CORES)
    xhi = nc.declare_dram_parameter("xhi", [B_PER, T, N], BF16, isOutput=False)
    xlo = nc.declare_dram_parameter("xlo", [B_PER, T, N], BF16, isOutput=False)
    lt = nc.declare_dram_parameter("lt", [P, 2 * P], BF16, isOutput=False)
    out = nc.declare_dram_parameter("out", [B_PER, T, N], F32, isOutput=True)

    with TileContext(nc) as tc:
        with (
            tc.tile_pool(name="wpool", bufs=1) as wpool,
            tc.tile_pool(name="xpool", bufs=4) as xpool,
            tc.tile_pool(name="opool", bufs=4) as opool,
            tc.tile_pool(name="pspool", bufs=4, space="PSUM") as pspool,
        ):
            w = wpool.tile([P, 2 * P], BF16)
            nc.sync.dma_start(out=w[:], in_=lt[:])
            wtri = w[:, 0:P]    # lhsT of L00 (== L11)
            w10 = w[:, P:2 * P]  # lhsT of L10

            for b in range(B_PER):
                # [256, N] time-major rows -> partition p holds rows p and p+128
                src_hi = xhi[b].rearrange("(k p) n -> p k n", p=P)
                src_lo = xlo[b].rearrange("(k p) n -> p k n", p=P)
                dst = out[b].rearrange("(k p) n -> p k n", p=P)
                for j in range(NJ):
                    nsl = slice(j * TK, (j + 1) * TK)
                    xh = xpool.tile([P, 2, TK], BF16, tag="xh")
                    xl = xpool.tile([P, 2, TK], BF16, tag="xl")
                    nc.sync.dma_start(out=xh[:], in_=src_hi[:, :, nsl])
                    nc.sync.dma_start(out=xl[:], in_=src_lo[:, :, nsl])
                    st = opool.tile([P, 2, TK], F32, tag="st")
                    for s in range(SL):
                        ssl = slice(s * MM, (s + 1) * MM)
                        p0 = pspool.tile([P, MM], F32, tag="p0")
                        p1 = pspool.tile([P, MM], F32, tag="p1")
                        # rows 0..127: L00 @ X0
                        nc.tensor.matmul(p0, wtri, xh[:, 0, ssl],
                                         start=True, stop=False)
                        nc.tensor.matmul(p0, wtri, xl[:, 0, ssl],
                                         start=False, stop=True)
                        # rows 128..255: L11 @ X1 + L10 @ X0
                        nc.tensor.matmul(p1, wtri, xh[:, 1, ssl],
                                         start=True, stop=False)
                        nc.tensor.matmul(p1, wtri, xl[:, 1, ssl],
                                         start=False, stop=False)
                        nc.tensor.matmul(p1, w10, xh[:, 0, ssl],
                                         start=False, stop=False)
                        nc.tensor.matmul(p1, w10, xl[:, 0, ssl],
                                         start=False, stop=True)
                        nc.vector.tensor_copy(st[:, 0, ssl], p0[:])
                        nc.vector.tensor_copy(st[:, 1, ssl], p1[:])
                    # second HWDGE ring (qActDynamicHW) — overlaps with the
                    # input stream on the SP ring instead of queueing behind it
                    nc.scalar.dma_start(out=dst[:, :, nsl], in_=st[:])
    nc.compile()
    return nc


def _get_nc():
    global _compiled_nc
    if _compiled_nc is None:
        _compiled_nc = _build_nc()
    return _compiled_nc


def _weights(a: float) -> np.ndarray:
    """lhsT blocks [wtri | w10] as [128, 256] bf16.

    wtri[k, m] = a^(m-k) for m >= k (transposed lower-tri block),
    w10[k, m]  = a^(128+m-k).
    """
    d = np.arange(P)
    e_tri = d[None, :] - d[:, None]           # m - k
    tri = np.where(e_tri >= 0, np.power(float(a), e_tri.clip(0)), 0.0)
    e_10 = 128 + d[None, :] - d[:, None]      # 128 + m - k
    blk10 = np.power(float(a), e_10)
    lt = np.concatenate([tri, blk10], axis=1).astype(np.float32)
    return lt.astype(ml_dtypes.bfloat16)


def _run(inputs: dict, trace: bool = False):
    X = np.asarray(inputs["X"], dtype=np.float32)
    w = np.asarray(inputs["w"], dtype=np.float32)
    assert X.shape == (B, T, N), X.shape

    a = 1.0 / (1.0 + math.exp(-float(w)))
    lt = _weights(a)

    x_hi = X.astype(ml_dtypes.bfloat16)
    x_lo = (X - x_hi.astype(np.float32)).astype(ml_dtypes.bfloat16)

    in_maps = []
    for i in range(N_CORES):
        sl = slice(i * B_PER, (i + 1) * B_PER)
        in_maps.append({"xhi": x_hi[sl], "xlo": x_lo[sl], "lt": lt})

    nc = _get_nc()
    r = run_bass_kernel_spmd(nc, in_maps, core_ids=list(range(N_CORES)),
                             trace=trace)
    out = np.concatenate([r.results[i]["out"] for i in range(N_CORES)], axis=0)
    return out, r


def kernel(**inputs) -> np.ndarray:
    out, _ = _run(inputs, trace=False)
    return out


# revision 16
# speedup vs baseline: 1.6738x; 1.6738x over previous
"""Leaky-integrator scan out[:,t] = out[:,t-1]*sigmoid(w) + X[:,t] on 8 trn2 cores.

Reformulated as a lower-triangular Toeplitz matmul over the time dim:
    out[b] = L @ X[b],  L[t, s] = a^(t-s) (t >= s),  a = sigmoid(w)
with T=256 split into two 128-row blocks. By Toeplitz structure L00 == L11
(lower-tri powers) and L10[i, j] = a^(128+i-j), so only two stationary
128x128 weight matrices are needed on the TensorEngine.

Numerics / HBM traffic (this is a memory-bound problem, so bytes == time):
  - input: X split on the host into fp16 hi + fp8e5m2 lo (3 B/elem); both
    halves accumulate into the same fp32 PSUM group (fp16/fp8 matmuls run
    at full PE rate), cancelling the fp16 input quantization.
  - output: stored fp16 (2 B/elem), upcast to f32 on the host.
  End-to-end error vs the f32 reference: ~2e-4 relative (dominated by the
  fp16 output rounding), with 80 MiB/core of HBM traffic instead of the
  128 MiB of a pure f32 pipeline.

Sharding: data-parallel over batch B (16 / 8 cores = 2 per core), which
keeps host-side shard slices contiguous and needs no cross-core traffic.

DMA: input stream issues on the SP HWDGE ring (nc.sync), output stream on
the ACT HWDGE ring (nc.scalar) — two hardware descriptor rings running
concurrently instead of one serialized queue.
"""

import math
import os
import sys

import numpy as np

for _p in ("/opt/trn_rl_repo", "/root/.axon_site/_ro/trn_rl_repo"):
    if os.path.isdir(_p) and _p not in sys.path:
        sys.path.insert(0, _p)

import ml_dtypes

import concourse.bass as bass
import concourse.mybir as mybir
from concourse import bacc
from concourse.tile import TileContext
from concourse.bass_utils import run_bass_kernel_spmd

B, T, N = 16, 256, 32768
N_CORES = 8
B_PER = B // N_CORES  # 2
P = 128               # partitions / time-block size
TK = 2048             # free-dim (feature) tile width
MM = 512              # matmul moving free dim (one PSUM bank of fp32)
NJ = N // TK          # 16 feature tiles per batch
SL = TK // MM         # 4 matmul slices per feature tile

FP16 = mybir.dt.float16
FP8 = mybir.dt.float8e5
F32 = mybir.dt.float32

_compiled_nc = None


def _build_nc():
    """Build + compile the SPMD Bass graph (identical on all 8 cores)."""
    nc = bacc.Bacc("TRN2", target_bir_lowering=False, debug=False,
                   num_devices=N_CORES)
    xhi = nc.declare_dram_parameter("xhi", [B_PER, T, N], FP16, isOutput=False)
    xlo = nc.declare_dram_parameter("xlo", [B_PER, T, N], FP8, isOutput=False)
    lt = nc.declare_dram_parameter("lt", [P, 2 * P], FP16, isOutput=False)
    lt8 = nc.declare_dram_parameter("lt8", [P, 2 * P], FP8, isOutput=False)
    out = nc.declare_dram_parameter("out", [B_PER, T, N], FP16, isOutput=True)

    with TileContext(nc) as tc:
        with (
            tc.tile_pool(name="wpool", bufs=1) as wpool,
            tc.tile_pool(name="xpool", bufs=3) as xpool,
            tc.tile_pool(name="opool", bufs=3) as opool,
            tc.tile_pool(name="pspool", bufs=4, space="PSUM") as pspool,
        ):
            # weights ride the ACT ring, which is idle until the first
            # output tile — keeps the SP ring free for the first inputs
            w = wpool.tile([P, 2 * P], FP16)
            nc.scalar.dma_start(out=w[:], in_=lt[:])
            wtri = w[:, 0:P]    # lhsT of L00 (== L11)
            w10 = w[:, P:2 * P]  # lhsT of L10
            w8 = wpool.tile([P, 2 * P], FP8)
            nc.scalar.dma_start(out=w8[:], in_=lt8[:])
            wtri8 = w8[:, 0:P]
            w10_8 = w8[:, P:2 * P]

            for b in range(B_PER):
                # [256, N] time-major rows -> partition p holds rows p and p+128
                src_hi = xhi[b].rearrange("(k p) n -> p k n", p=P)
                src_lo = xlo[b].rearrange("(k p) n -> p k n", p=P)
                dst = out[b].rearrange("(k p) n -> p k n", p=P)
                for j in range(NJ):
                    nsl = slice(j * TK, (j + 1) * TK)
                    xh = xpool.tile([P, 2, TK], FP16, tag="xh")
                    xl = xpool.tile([P, 2, TK], FP8, tag="xl")
                    nc.sync.dma_start(out=xh[:], in_=src_hi[:, :, nsl])
                    # third DMA ring (SWDGE) for the fp8 lo stream
                    nc.gpsimd.dma_start(out=xl[:], in_=src_lo[:, :, nsl])
                    st = opool.tile([P, 2, TK], FP16, tag="st")
                    for s in range(SL):
                        ssl = slice(s * MM, (s + 1) * MM)
                        p0 = pspool.tile([P, MM], F32, tag="p0")
                        p1 = pspool.tile([P, MM], F32, tag="p1")
                        # rows 0..127: L00 @ X0 (fp16 hi + fp8 lo correction)
                        nc.tensor.matmul(p0, wtri, xh[:, 0, ssl],
                                         start=True, stop=False)
                        nc.tensor.matmul(p0, wtri8, xl[:, 0, ssl],
                                         start=False, stop=True)
                        # rows 128..255: L11 @ X1 + L10 @ X0
                        nc.tensor.matmul(p1, wtri, xh[:, 1, ssl],
                                         start=True, stop=False)
                        nc.tensor.matmul(p1, wtri8, xl[:, 1, ssl],
                                         start=False, stop=False)
                        nc.tensor.matmul(p1, w10, xh[:, 0, ssl],
                                         start=False, stop=False)
                        nc.tensor.matmul(p1, w10_8, xl[:, 0, ssl],
                                         start=False, stop=True)
                        nc.vector.tensor_copy(st[:, 0, ssl], p0[:])
                        nc.vector.tensor_copy(st[:, 1, ssl], p1[:])
                    # second HWDGE ring (qActDynamicHW) — overlaps with the
                    # input stream on the SP ring instead of queueing behind it
                    nc.scalar.dma_start(out=dst[:, :, nsl], in_=st[:])
    nc.compile()
    return nc


def _get_nc():
    global _compiled_nc
    if _compiled_nc is None:
        _compiled_nc = _build_nc()
    return _compiled_nc


def _weights(a: float) -> np.ndarray:
    """lhsT blocks [wtri | w10] as [128, 256] f32.

    wtri[k, m] = a^(m-k) for m >= k (transposed lower-tri block),
    w10[k, m]  = a^(128+m-k).
    """
    d = np.arange(P)
    e_tri = d[None, :] - d[:, None]           # m - k
    tri = np.where(e_tri >= 0, np.power(float(a), e_tri.clip(0)), 0.0)
    e_10 = 128 + d[None, :] - d[:, None]      # 128 + m - k
    blk10 = np.power(float(a), e_10)
    return np.concatenate([tri, blk10], axis=1).astype(np.float32)


def _run(inputs: dict, trace: bool = False):
    X = np.asarray(inputs["X"], dtype=np.float32)
    w = np.asarray(inputs["w"], dtype=np.float32)
    assert X.shape == (B, T, N), X.shape

    a = 1.0 / (1.0 + math.exp(-float(w)))
    ltf = _weights(a)
    lt = ltf.astype(np.float16)
    lt8 = ltf.astype(ml_dtypes.float8_e5m2)

    x_hi = X.astype(np.float16)
    x_lo = (X - x_hi.astype(np.float32)).astype(ml_dtypes.float8_e5m2)

    in_maps = []
    for i in range(N_CORES):
        sl = slice(i * B_PER, (i + 1) * B_PER)
        in_maps.append({"xhi": x_hi[sl], "xlo": x_lo[sl], "lt": lt,
                        "lt8": lt8})

    nc = _get_nc()
    r = run_bass_kernel_spmd(nc, in_maps, core_ids=list(range(N_CORES)),
                             trace=trace)
    out = np.concatenate([r.results[i]["out"] for i in range(N_CORES)],
                         axis=0).astype(np.float32)
    return out, r


def kernel(**inputs) -> np.ndarray:
    out, _ = _run(inputs, trace=False)
    return out


# revision 23
# speedup vs baseline: 1.9565x; 1.1689x over previous
"""Leaky-integrator scan out[:,t] = out[:,t-1]*sigmoid(w) + X[:,t] on 8 trn2 cores.

Reformulated as a lower-triangular Toeplitz matmul over the time dim:
    out[b] = L @ X[b],  L[t, s] = a^(t-s) (t >= s),  a = sigmoid(w)
with T=256 split into two 128-row blocks. By Toeplitz structure L00 == L11
(lower-tri powers) and L10[i, j] = a^(128+i-j), so only two stationary
128x128 weight matrices are needed on the TensorEngine.

Numerics / HBM traffic (this is a memory-bound problem, so bytes == time):
  - input: fp16 (2 B/elem), matmul at full PE rate with fp32 PSUM accum
  - output: stored fp16 (2 B/elem), upcast to f32 on the host
  64 MiB/core of HBM traffic instead of the 128 MiB of a pure f32
  pipeline; end-to-end error vs the f32 reference ~3e-4 relative (fp16
  input+output rounding, both with 10-bit mantissas).

Sharding: data-parallel over batch B (16 / 8 cores = 2 per core), which
keeps host-side shard slices contiguous and needs no cross-core traffic.

DMA: the input stream alternates between the SP HWDGE ring (nc.sync) and
the SWDGE ring (nc.gpsimd); the output stream rides the ACT HWDGE ring
(nc.scalar) — three descriptor rings running concurrently instead of one
serialized queue.
"""

import math
import os
import sys

import numpy as np

for _p in ("/opt/trn_rl_repo", "/root/.axon_site/_ro/trn_rl_repo"):
    if os.path.isdir(_p) and _p not in sys.path:
        sys.path.insert(0, _p)

import ml_dtypes

import concourse.bass as bass
import concourse.mybir as mybir
from concourse import bacc
from concourse.tile import TileContext
from concourse.bass_utils import run_bass_kernel_spmd

B, T, N = 16, 256, 32768
N_CORES = 8
B_PER = B // N_CORES  # 2
P = 128               # partitions / time-block size
TK = 4096             # free-dim (feature) tile width
MM = 512              # matmul moving free dim (one PSUM bank of fp32)
NJ = N // TK          # feature tiles per batch
SL = TK // MM         # matmul slices per feature tile

FP16 = mybir.dt.float16
F32 = mybir.dt.float32

_compiled_nc = None


def _build_nc():
    """Build + compile the SPMD Bass graph (identical on all 8 cores)."""
    nc = bacc.Bacc("TRN2", target_bir_lowering=False, debug=False,
                   num_devices=N_CORES)
    xhi = nc.declare_dram_parameter("xhi", [B_PER, T, N], FP16, isOutput=False)
    lt = nc.declare_dram_parameter("lt", [P, 2 * P], FP16, isOutput=False)
    out = nc.declare_dram_parameter("out", [B_PER, T, N], FP16, isOutput=True)

    with TileContext(nc) as tc:
        with (
            tc.tile_pool(name="wpool", bufs=1) as wpool,
            tc.tile_pool(name="xpool", bufs=3) as xpool,
            tc.tile_pool(name="opool", bufs=3) as opool,
            tc.tile_pool(name="pspool", bufs=4, space="PSUM") as pspool,
        ):
            # weights ride the ACT ring, which is idle until the first
            # output tile — keeps the SP ring free for the first inputs
            w = wpool.tile([P, 2 * P], FP16)
            nc.scalar.dma_start(out=w[:], in_=lt[:])
            wtri = w[:, 0:P]    # lhsT of L00 (== L11)
            w10 = w[:, P:2 * P]  # lhsT of L10

            for b in range(B_PER):
                # [256, N] time-major rows -> partition p holds rows p and p+128
                src_hi = xhi[b].rearrange("(k p) n -> p k n", p=P)
                dst = out[b].rearrange("(k p) n -> p k n", p=P)
                for j in range(NJ):
                    nsl = slice(j * TK, (j + 1) * TK)
                    xh = xpool.tile([P, 2, TK], FP16, tag="xh")
                    # alternate the input stream across the SP HWDGE ring
                    # and the SWDGE ring so two rings carry it concurrently
                    dma_in = nc.sync if (b * NJ + j) % 2 == 0 else nc.gpsimd
                    dma_in.dma_start(out=xh[:], in_=src_hi[:, :, nsl])
                    st = opool.tile([P, 2, TK], FP16, tag="st")
                    for s in range(SL):
                        ssl = slice(s * MM, (s + 1) * MM)
                        p0 = pspool.tile([P, MM], F32, tag="p0")
                        p1 = pspool.tile([P, MM], F32, tag="p1")
                        # rows 0..127: L00 @ X0
                        nc.tensor.matmul(p0, wtri, xh[:, 0, ssl],
                                         start=True, stop=True)
                        # rows 128..255: L11 @ X1 + L10 @ X0
                        nc.tensor.matmul(p1, wtri, xh[:, 1, ssl],
                                         start=True, stop=False)
                        nc.tensor.matmul(p1, w10, xh[:, 0, ssl],
                                         start=False, stop=True)
                        nc.vector.tensor_copy(st[:, 0, ssl], p0[:])
                        nc.vector.tensor_copy(st[:, 1, ssl], p1[:])
                    # second HWDGE ring (qActDynamicHW) — overlaps with the
                    # input stream instead of queueing behind it
                    nc.scalar.dma_start(out=dst[:, :, nsl], in_=st[:])
    nc.compile()
    return nc


def _get_nc():
    global _compiled_nc
    if _compiled_nc is None:
        _compiled_nc = _build_nc()
    return _compiled_nc


def _weights(a: float) -> np.ndarray:
    """lhsT blocks [wtri | w10] as [128, 256] f32.

    wtri[k, m] = a^(m-k) for m >= k (transposed lower-tri block),
    w10[k, m]  = a^(128+m-k).
    """
    d = np.arange(P)
    e_tri = d[None, :] - d[:, None]           # m - k
    tri = np.where(e_tri >= 0, np.power(float(a), e_tri.clip(0)), 0.0)
    e_10 = 128 + d[None, :] - d[:, None]      # 128 + m - k
    blk10 = np.power(float(a), e_10)
    return np.concatenate([tri, blk10], axis=1).astype(np.float32)


def _run(inputs: dict, trace: bool = False):
    X = np.asarray(inputs["X"], dtype=np.float32)
    w = np.asarray(inputs["w"], dtype=np.float32)
    assert X.shape == (B, T, N), X.shape

    a = 1.0 / (1.0 + math.exp(-float(w)))
    lt = _weights(a).astype(np.float16)

    x_hi = X.astype(np.float16)

    in_maps = []
    for i in range(N_CORES):
        sl = slice(i * B_PER, (i + 1) * B_PER)
        in_maps.append({"xhi": x_hi[sl], "lt": lt})

    nc = _get_nc()
    r = run_bass_kernel_spmd(nc, in_maps, core_ids=list(range(N_CORES)),
                             trace=trace)
    out = np.concatenate([r.results[i]["out"] for i in range(N_CORES)],
                         axis=0).astype(np.float32)
    return out, r


def kernel(**inputs) -> np.ndarray:
    out, _ = _run(inputs, trace=False)
    return out
